# revision 1
# baseline (speedup 1.0000x reference)
"""KStoNet (RBF-SVR heads + MLP) fused Trainium2 kernel, data-parallel over 8 cores."""
import sys

sys.path.insert(0, "/opt/trn_rl_repo")

import contextlib
import ctypes
import types

import numpy as np


def _install_axon_shims():
    """(1) NTFF profile hook this image's antenv lacks; (2) split the final SP
    Drain's sem waits (this walrus build allows only one sync wait there)."""
    if "antenv.axon_hooks" not in sys.modules:
        lib = ctypes.CDLL("/opt/axon/libaxon_pjrt.so")
        hook = None
        if hasattr(lib, "axon_start_nrt_profile"):
            lib.axon_start_nrt_profile.argtypes = [
                ctypes.POINTER(ctypes.c_int64),
                ctypes.c_size_t,
            ]
            lib.axon_start_nrt_profile.restype = ctypes.c_int64
            lib.axon_stop_nrt_profile.argtypes = [ctypes.c_char_p]
            lib.axon_stop_nrt_profile.restype = ctypes.c_int64

            @contextlib.contextmanager
            def _hook(output_dir, device_ids=None):
                import jax

                jax.devices()
                if device_ids:
                    ids = (ctypes.c_int64 * len(device_ids))(*device_ids)
                    rc = lib.axon_start_nrt_profile(ids, len(device_ids))
                else:
                    rc = lib.axon_start_nrt_profile(None, 0)
                if rc != 0:
                    raise RuntimeError(f"axon_start_nrt_profile rc={rc}")
                try:
                    yield
                finally:
                    n = lib.axon_stop_nrt_profile(str(output_dir).encode())
                    print(f"profile: {n} file(s) -> {output_dir}", file=sys.stderr)

            hook = _hook
        mod = types.ModuleType("antenv.axon_hooks")
        mod.get_axon_ntff_profile_hook = lambda: hook
        mod.set_axon_ntff_profile_hook = lambda h: None
        sys.modules["antenv.axon_hooks"] = mod
        import antenv

        antenv.axon_hooks = mod

    import bass_rust
    import concourse.tile as tile
    from concourse.vector_clock import ScopedClock

    if not getattr(tile.TileContext._drain_and_barrier, "_wait_split", False):

        def _drain_and_barrier(self, tick_clock, wait_clock):
            drain_inst = self.nc.sync.drain()
            wait_clock.add_sem_waits(
                drain_inst.ins, ScopedClock({None: tick_clock.global_clock})
            )
            si = drain_inst.ins.sync_info
            waits = list(si.on_wait) if si and si.on_wait else []
            if len(waits) > 1:
                si.on_wait = waits[:1]
                for w in waits[1:]:
                    extra = self.nc.sync.drain()
                    extra.ins.sync_info = bass_rust.SyncInfo(on_wait=[w], on_update=[])
            self.nc.all_engine_barrier()
            assert self.sems is not None
            popped = self.nc._tile_sem_poison_stack.pop()
            assert popped is self._sem_poison
            self.nc.clear_and_free_semaphores(list(self.sems.allocated().values()))
            self.nc.all_engine_barrier()

        _drain_and_barrier._wait_split = True
        tile.TileContext._drain_and_barrier = _drain_and_barrier


_install_axon_shims()

import ml_dtypes
import concourse.bass as bass
import concourse.tile as tile
from concourse import bacc, mybir
from concourse.bass_utils import run_bass_kernel_spmd

GAMMA = 0.1
B, D, H0, K = 16384, 64, 256, 50
HK = H0 * K  # 12800
NCORES = 8
BC = B // NCORES  # 2048 batch rows per core
SLAB = 512  # psum-bank width of one batch slab
NSLAB = BC // SLAB
NCHUNK = HK // 128  # 100 chunks of 128 (head,k) pairs
GROUP = 3  # chunks per Exp read (3 psum banks)
BF16 = mybir.dt.bfloat16
F32 = mybir.dt.float32

_CACHE = {}


def _build_program():
    nc = bacc.Bacc("TRN2", target_bir_lowering=False, debug=False)
    xaugT_d = nc.dram_tensor("xaugT", [D + 2, BC], BF16, kind="ExternalInput")
    caugT_d = nc.dram_tensor("caugT", [D + 2, HK], BF16, kind="ExternalInput")
    wm_d = nc.dram_tensor("wm", [128, NCHUNK * 128], BF16, kind="ExternalInput")
    svrb_d = nc.dram_tensor("svrb", [128, 2], F32, kind="ExternalInput")
    fcb_d = nc.dram_tensor("fcb", [128, 2], F32, kind="ExternalInput")
    fcT_d = nc.dram_tensor("fcT", [H0, H0], F32, kind="ExternalInput")
    owT_d = nc.dram_tensor("owT", [H0, 1], F32, kind="ExternalInput")
    out_d = nc.dram_tensor("out", [BC], F32, kind="ExternalOutput")

    Exp = mybir.ActivationFunctionType.Exp
    Tanh = mybir.ActivationFunctionType.Tanh
    Copy = mybir.ActivationFunctionType.Copy

    with tile.TileContext(nc) as tc:
        with (
            tc.tile_pool(name="const", bufs=1) as constp,
            tc.tile_pool(name="cw", bufs=1) as cwp,
            tc.tile_pool(name="rbfw", bufs=4) as rbfwp,
            tc.tile_pool(name="hid", bufs=2) as hidp,
            tc.tile_pool(name="orow", bufs=2) as orowp,
            tc.tile_pool(name="pt1", bufs=2, space=bass.MemorySpace.PSUM) as pt1p,
            tc.tile_pool(name="pacc", bufs=1, space=bass.MemorySpace.PSUM) as paccp,
            tc.tile_pool(name="p34", bufs=1, space=bass.MemorySpace.PSUM) as p34p,
        ):
            # ---- constant loads ----
            xaug_sb = constp.tile([D + 2, BC], BF16, tag="xaug")
            nc.sync.dma_start(xaug_sb[:], xaugT_d.ap())
            # caug/wm DMA pieces; tiny piece 0 so chunk 0 starts ASAP
            PIECES = [2, 8, 18, 18, 18, 18, 18]  # chunks per piece, sums to 100
            piece_of = []
            for i, npc in enumerate(PIECES):
                for j in range(npc):
                    piece_of.append((i, j))
            caug_sb = []
            wm_sb = []
            coff = 0
            for i, npc in enumerate(PIECES):
                ct = cwp.tile([D + 2, npc * 128], BF16, tag=f"caug{i}", name=f"caug{i}")
                nc.sync.dma_start(
                    ct[:], caugT_d.ap()[:, coff * 128 : (coff + npc) * 128]
                )
                caug_sb.append(ct)
                wt = cwp.tile([128, npc * 128], BF16, tag=f"wm{i}", name=f"wm{i}")
                nc.sync.dma_start(
                    wt[:], wm_d.ap()[:, coff * 128 : (coff + npc) * 128]
                )
                wm_sb.append(wt)
                coff += npc
            svrb_sb = constp.tile([128, 2], F32, tag="svrb")
            nc.sync.dma_start(svrb_sb[:], svrb_d.ap())
            fcb_sb = constp.tile([128, 2], F32, tag="fcb")
            nc.sync.dma_start(fcb_sb[:], fcb_d.ap())
            fcT_sb = []
            for hh in range(2):
                ft = constp.tile([128, H0], F32, tag=f"fcT{hh}")
                nc.sync.dma_start(ft[:], fcT_d.ap()[hh * 128 : (hh + 1) * 128, :])
                fcT_sb.append(ft)
            owT_sb = []
            for hh in range(2):
                ot = constp.tile([128, 1], F32, tag=f"owT{hh}")
                nc.sync.dma_start(ot[:], owT_d.ap()[hh * 128 : (hh + 1) * 128, :])
                owT_sb.append(ot)

            def caug_ap(c):
                i, j = piece_of[c]
                return caug_sb[i][:, j * 128 : (j + 1) * 128]

            def wm_ap(c):
                i, j = piece_of[c]
                return wm_sb[i][:, j * 128 : (j + 1) * 128]

            # ---- main loop ----
            for s in range(NSLAB):
                xslab = xaug_sb[:, s * SLAB : (s + 1) * SLAB]
                acc = [None, None]
                hidT = [None, None]
                ntri = (NCHUNK + GROUP - 1) // GROUP
                for t in range(ntri):
                    cs = [c for c in range(t * GROUP, (t + 1) * GROUP) if c < NCHUNK]
                    pt1 = pt1p.tile([128, SLAB * GROUP], F32, tag="pt1")
                    for j, c in enumerate(cs):
                        nc.tensor.matmul(
                            pt1[:, j * SLAB : (j + 1) * SLAB],
                            caug_ap(c),
                            xslab,
                            start=True,
                            stop=True,
                        )
                    rb = rbfwp.tile([128, SLAB * GROUP], BF16, tag="rb")
                    ncols = len(cs) * SLAB
                    nc.scalar.activation(rb[:, :ncols], pt1[:, :ncols], Exp)
                    for j, c in enumerate(cs):
                        half = c // (NCHUNK // 2)
                        if c % (NCHUNK // 2) == 0:
                            acc[half] = paccp.tile([128, SLAB], F32, tag="acc", name=f"acc{half}")
                        nc.tensor.matmul(
                            acc[half][:],
                            wm_ap(c),
                            rb[:, j * SLAB : (j + 1) * SLAB],
                            start=(c % (NCHUNK // 2) == 0),
                            stop=(c % (NCHUNK // 2) == (NCHUNK // 2 - 1)),
                        )
                        if c % (NCHUNK // 2) == NCHUNK // 2 - 1:
                            ht = hidp.tile([128, SLAB], F32, tag="hidT")
                            nc.scalar.activation(
                                ht[:],
                                acc[half][:],
                                Tanh,
                                bias=svrb_sb[:, half : half + 1],
                            )
                            hidT[half] = ht
                # stage 3: hidden2T = tanh(fcT.T-blocks @ hidT + fcb)
                h2T = [None, None]
                for jh in range(2):
                    psB = p34p.tile([128, SLAB], F32, tag="p34")
                    for hh in range(2):
                        nc.tensor.matmul(
                            psB[:],
                            fcT_sb[hh][:, jh * 128 : (jh + 1) * 128],
                            hidT[hh][:],
                            start=(hh == 0),
                            stop=(hh == 1),
                        )
                    h2 = hidp.tile([128, SLAB], F32, tag="h2T")
                    nc.scalar.activation(
                        h2[:], psB[:], Tanh, bias=fcb_sb[:, jh : jh + 1]
                    )
                    h2T[jh] = h2
                # stage 4: out = owT.T @ h2T + out_b  (out_b added on host)
                psC = p34p.tile([1, SLAB], F32, tag="p34", name="psC")
                for jh in range(2):
                    nc.tensor.matmul(
                        psC[:],
                        owT_sb[jh][:],
                        h2T[jh][:],
                        start=(jh == 0),
                        stop=(jh == 1),
                    )
                orow = orowp.tile([1, SLAB], F32, tag="orow")
                nc.vector.tensor_copy(orow[:], psC[:])
                nc.sync.dma_start(out_d.ap()[s * SLAB : (s + 1) * SLAB], orow[0:1, :])
    nc.compile()
    return nc


def _prep_inputs(x, centers, svr_w, svr_b, fc_w, fc_b, out_w, out_b):
    bf16 = ml_dtypes.bfloat16
    x = np.asarray(x, np.float32)
    centers = np.asarray(centers, np.float32)
    # xaugT: rows 0..63 = x.T; 64,65 = hi/lo split of -gamma*|x|^2
    x2 = (x * x).sum(-1)  # [B]
    t = (-GAMMA * x2).astype(np.float32)
    hi = t.astype(bf16)
    lo = (t - hi.astype(np.float32)).astype(bf16)
    xaugT = np.empty((D + 2, B), bf16)
    xaugT[:D] = x.T.astype(bf16)
    xaugT[D] = hi
    xaugT[D + 1] = lo
    # caugT: rows 0..63 = 2*gamma*centers[hk,d] transposed; 64,65 = 1
    cfl = centers.reshape(HK, D)
    caugT = np.empty((D + 2, HK), bf16)
    caugT[:D] = (2.0 * GAMMA * cfl).T.astype(bf16)
    caugT[D] = bf16(1.0)
    caugT[D + 1] = bf16(1.0)
    # stage-2 weights: wm[p, c*128 + h] = svr_w[h,k]*exp(-gamma*c2[h,k])
    # for hk = 128*c + p mapping to (h = hk//K, k = hk%K), else 0.
    c2 = (cfl * cfl).sum(-1)  # [HK]
    wfold = (np.asarray(svr_w, np.float32).reshape(HK) * np.exp(-GAMMA * c2)).astype(
        np.float32
    )
    hk = np.arange(HK)
    heads = hk // K  # global head of each hk
    wm = np.zeros((128, NCHUNK * 128), np.float32)
    p = hk % 128
    chunk = hk // 128
    col = chunk * 128 + (heads % 128)
    wm[p, col] = wfold
    wm = wm.astype(bf16)
    svrb = np.stack(
        [np.asarray(svr_b, np.float32)[:128], np.asarray(svr_b, np.float32)[128:]], 1
    )
    fcb = np.stack(
        [np.asarray(fc_b, np.float32)[:128], np.asarray(fc_b, np.float32)[128:]], 1
    )
    fcT = np.ascontiguousarray(np.asarray(fc_w, np.float32).T)  # [h, j]
    owT = np.ascontiguousarray(np.asarray(out_w, np.float32).T)  # [h, 1]
    return xaugT, caugT, wm, svrb, fcb, fcT, owT, float(np.asarray(out_b)[0])


def kernel(x, centers, svr_w, svr_b, fc_w, fc_b, out_w, out_b, _trace=False):
    if "nc" not in _CACHE:
        _CACHE["nc"] = _build_program()
    nc = _CACHE["nc"]
    xaugT, caugT, wm, svrb, fcb, fcT, owT, ob = _prep_inputs(
        x, centers, svr_w, svr_b, fc_w, fc_b, out_w, out_b
    )
    in_maps = []
    for c in range(NCORES):
        in_maps.append(
            {
                "xaugT": np.ascontiguousarray(xaugT[:, c * BC : (c + 1) * BC]),
                "caugT": caugT,
                "wm": wm,
                "svrb": svrb,
                "fcb": fcb,
                "fcT": fcT,
                "owT": owT,
            }
        )
    res = run_bass_kernel_spmd(nc, in_maps, list(range(NCORES)), trace=_trace)
    out = np.concatenate([res.results[c]["out"] for c in range(NCORES)])
    out = (out + ob).astype(np.float32).reshape(B, 1)
    if _trace:
        kernel._last_results = res
    return out

